# revision 41
# baseline (speedup 1.0000x reference)
"""DropConnect dense MLP kernel for Trainium2 (8 NeuronCores, data-parallel).

Computes y[b,o] = sum_i x[b,i] * w[i,o] * m[b,i,o] + bias[o]  where
m = jax.random.bernoulli(jax.random.key(42), 0.5, (128, 1024, 1024)).

The Bernoulli mask is a *constant* of the function (fixed key, fixed shape,
independent of all inputs): it is materialized once with the exact same
jax.random.bernoulli call the reference makes (the stream is backend-dependent
-- rbg/RngBitGenerator -- so the call is made on the default backend, exactly
like the reference) and shipped to the device as int8 {0,1} planes.

Device work per core (16 samples, data-parallel over batch):
  - wm = w * m for 4 samples per DVE tensor_tensor (f32 * int8 -> f32,
    w read through a step-0 broadcast AP)
  - y contributions via PE matmuls: per (sample, k-chunk) a [128,4] zero
    padded lhsT (x chunk in column sample//4) against wm, accumulated in
    PSUM. Samples rotate over the four 32-column PE groups
    (tile_position=(0,32j), j = sample%4) so the fp32 LOW_HIGH streams of
    4 samples overlap in the systolic array.
  - bias added with K=1 matmuls of ones[1,4].T @ bias[1,512] per group.
"""

import numpy as np

B, IN, OUT = 128, 1024, 1024
N_CORES = 8
BPC = B // N_CORES          # 16 samples per core
KCH = IN // 128             # 8 contraction chunks
NQ = BPC // 4               # 4 sample-quads per core

DROP_PROB = 0.5  # prob a weight is KEPT, matching the reference

# Route mask-multiply tiles with index % GPSIMD_MOD == 1 to GPSIMD (POOL).
# Measured SLOWER than DVE-only (SBUF port contention); 0 disables.
GPSIMD_MOD = 0

_mask_cache = None


def _dropconnect_mask_u8():
    """The constant keep-mask as uint8 {0,1}, shape (B, IN, OUT).

    Must match jax.random.bernoulli(jax.random.key(42), ...) bit-for-bit.
    The default PRNG impl here is 'rbg' (XLA RngBitGenerator), whose stream
    is backend-dependent, so this makes the *identical* call the reference
    makes, with no device pinning.
    """
    global _mask_cache
    if _mask_cache is None:
        import jax
        m = jax.random.bernoulli(jax.random.key(42), DROP_PROB, (B, IN, OUT))
        _mask_cache = np.asarray(m).astype(np.uint8)
    return _mask_cache


def _build_bass():
    import concourse.bacc as bacc
    import concourse.bass as bass
    import concourse.mybir as mybir
    import concourse.tile as tile

    nc = bacc.Bacc("TRN2", target_bir_lowering=False, debug=False,
                   num_devices=N_CORES)

    w_d = nc.dram_tensor("w", [KCH, 128, OUT], mybir.dt.float32,
                         kind="ExternalInput")
    xblk_d = nc.dram_tensor("xblk", [128, KCH * BPC * 4], mybir.dt.float32,
                            kind="ExternalInput")
    m8_d = nc.dram_tensor("m8", [KCH, 128, BPC * OUT], mybir.dt.int8,
                          kind="ExternalInput")
    bias_d = nc.dram_tensor("bias", [1, OUT], mybir.dt.float32,
                            kind="ExternalInput")
    ones_d = nc.dram_tensor("ones", [1, 4], mybir.dt.float32,
                            kind="ExternalInput")
    y_d = nc.dram_tensor("y", [BPC, OUT], mybir.dt.float32,
                         kind="ExternalOutput")
    y_ap = y_d.ap().rearrange("(q j) o -> j q o", j=4)

    with tile.TileContext(nc) as tc:
        with (
            tc.tile_pool(name="const", bufs=1) as cpool,
            tc.tile_pool(name="mstream", bufs=4) as mpool,
            tc.tile_pool(name="wm", bufs=5) as wmpool,
            tc.tile_pool(name="out", bufs=1) as opool,
            tc.tile_pool(name="psum", bufs=1, space="PSUM") as ppool,
        ):
            # first mask quad leads on the sync HWDGE ring so TT0 starts
            # early; small constants slot in behind it; W on the ACT ring.
            QSZ = 4 * OUT  # one quad of int8 mask
            # k=0 mask pieces alternate between the sync HWDGE ring and the
            # gpsimd SWDGE ring so the early loads transfer in parallel;
            # small leading pieces cut the first TT's dependency.
            m0_pieces = []  # (b0, nsamp, tile)
            k0_splits = [(0, 4), (4, 4), (8, 4), (12, 4)]
            for idx, (b0, ns) in enumerate(k0_splits):
                m0 = cpool.tile([128, ns * OUT], mybir.dt.int8,
                                name=f"m0p{idx}", tag=f"m0p{idx}")
                nc.sync.dma_start(out=m0[:],
                                  in_=m8_d[0][:, b0 * OUT:(b0 + ns) * OUT])
                m0_pieces.append((b0, ns, m0))
            # xblk/bias/ones ride behind the k=0 mask pieces: the DVE must
            # never wait, while the PE has ~100us of slack to absorb the lag
            xblk_t = cpool.tile([128, KCH * BPC * 4], mybir.dt.float32,
                                tag="xblk")
            nc.sync.dma_start(out=xblk_t[:], in_=xblk_d[:])
            bias_t = cpool.tile([1, OUT], mybir.dt.float32, tag="bias")
            nc.sync.dma_start(out=bias_t[:], in_=bias_d[:])
            ones_t = cpool.tile([1, 4], mybir.dt.float32, tag="ones")
            nc.sync.dma_start(out=ones_t[:], in_=ones_d[:])
            w_ts = []
            for k in range(KCH):
                w_t = cpool.tile([128, OUT], mybir.dt.float32, tag=f"w{k}")
                nc.scalar.dma_start(out=w_t[:], in_=w_d[k])
                w_ts.append(w_t)

            psum_ts = [ppool.tile([128, 512], mybir.dt.float32,
                                  name=f"ps{h}", tag=f"ps{h}")
                       for h in range(2)]
            # zero-fill so the full-width tail copies read only initialized
            # PSUM; runs during the DMA ramp while the DVE is idle
            for h in range(2):
                nc.vector.memset(psum_ts[h][:], 0.0)

            def emit_tile(k, b0, ns, m_ap, g):
                """Mask-multiply ns samples [b0, b0+ns) and their matmuls."""
                wm_t = wmpool.tile([128, ns * OUT], mybir.dt.float32,
                                   name="wm", tag="wm", padded_shape=None)
                w_sl = w_ts[k][:]
                w_bc = bass.AP(
                    tensor=w_sl.tensor, offset=w_sl.offset,
                    ap=[w_sl.ap[0], [0, ns], w_sl.ap[1]])
                eng = (nc.gpsimd if GPSIMD_MOD and g % GPSIMD_MOD == 1
                       else nc.vector)
                eng.tensor_tensor(
                    wm_t[:, :ns * OUT].rearrange("p (i o) -> p i o", i=ns),
                    w_bc,
                    m_ap.rearrange("p (i o) -> p i o", i=ns),
                    mybir.AluOpType.mult)
                for half in range(2):
                    for i in range(ns):
                        b = b0 + i
                        j = b % 4
                        lhsT = xblk_t[:, (k * BPC + b) * 4:
                                      (k * BPC + b + 1) * 4]
                        rhs = wm_t[:, i * OUT + half * 512:
                                   i * OUT + half * 512 + 512]
                        psl = psum_ts[half][32 * j:32 * j + 4, :]
                        nc.tensor.matmul(psl, lhsT, rhs,
                                         start=(k == 0 and b < 4),
                                         stop=(k == KCH - 1 and b >= BPC - 4),
                                         tile_position=(0, 32 * j))
                if k == 0 and b0 + ns == 4:
                    # all four PE groups are started: fold bias in now
                    for half in range(2):
                        for j in range(4):
                            psl = psum_ts[half][32 * j:32 * j + 4, :]
                            nc.tensor.matmul(
                                psl, ones_t[:],
                                bias_t[:, half * 512:half * 512 + 512],
                                start=False, stop=False,
                                tile_position=(0, 32 * j))

            g = 0
            for b0, ns, m0 in m0_pieces:
                emit_tile(0, b0, ns, m0[:], g)
                g += 1
            # k >= 1: m8 streamed in 1MB halves (2 quad-TTs each), halves
            # alternating across the two ring families for bandwidth headroom
            for k in range(1, KCH):
                for h in range(2):
                    m_t = mpool.tile([128, BPC * OUT // 2], mybir.dt.int8,
                                     tag="m")
                    nc.sync.dma_start(
                        out=m_t[:],
                        in_=m8_d[k][:, h * (BPC * OUT // 2):
                                    (h + 1) * (BPC * OUT // 2)])
                    if k == KCH - 1 and h == 1:
                        # final tiles shrink to single samples: less PE work
                        # exposed after the last DVE op, shorter drain
                        emit_tile(k, 8, 4, m_t[:, 0:QSZ], g)
                        g += 1
                        for i in range(4):
                            emit_tile(k, 12 + i, 1,
                                      m_t[:, QSZ + i * OUT:
                                           QSZ + (i + 1) * OUT], g)
                            g += 1
                    else:
                        for mq in range(2):
                            emit_tile(k, (h * 2 + mq) * 4, 4,
                                      m_t[:, mq * QSZ:(mq + 1) * QSZ], g)
                            g += 1

            # one full-width copy per half (garbage partitions copied too --
            # harmless; only valid rows are DMA'd out). ACT and DVE halves
            # read disjoint PSUM banks and run in parallel.
            ysb_t = opool.tile([128, OUT], mybir.dt.float32, tag="ysb")
            nc.scalar.copy(ysb_t[:, 0:512], psum_ts[0][:])
            nc.vector.tensor_copy(ysb_t[:, 512:1024], psum_ts[1][:])
            for j in range(4):
                ring = nc.sync if j % 2 == 0 else nc.scalar
                ring.dma_start(out=y_ap[j],
                               in_=ysb_t[32 * j:32 * j + 4, :])

    nc.compile()
    return nc


def _host_inputs(x, weight, bias):
    """Per-core input maps (layouts are pure data movement + zero padding)."""
    x = np.asarray(x, np.float32)
    weight = np.asarray(weight, np.float32)
    bias = np.asarray(bias, np.float32).reshape(1, OUT)

    mask = _dropconnect_mask_u8()

    # weight: [IN, OUT] -> [KCH, 128, OUT]
    w_host = np.ascontiguousarray(weight.reshape(KCH, 128, OUT))
    ones = np.ones((1, 4), np.float32)

    in_maps = []
    for c in range(N_CORES):
        xs = x[c * BPC:(c + 1) * BPC]  # [BPC, IN]
        xblk = np.zeros((128, KCH, BPC, 4), np.float32)
        for k in range(KCH):
            for b in range(BPC):
                xblk[:, k, b, b // 4] = xs[b, k * 128:(k + 1) * 128]
        xblk = xblk.reshape(128, KCH * BPC * 4)

        msk = mask[c * BPC:(c + 1) * BPC]  # [BPC, IN, OUT] uint8
        m8 = np.ascontiguousarray(
            msk.reshape(BPC, KCH, 128, OUT).transpose(1, 2, 0, 3)
        ).reshape(KCH, 128, BPC * OUT).view(np.int8)

        in_maps.append({
            "w": w_host, "xblk": xblk, "m8": m8,
            "bias": bias, "ones": ones,
        })
    return in_maps


def _gather_output(results):
    # DRAM row q*4+j holds sample b = 4q+j == the row index itself: identity.
    return np.concatenate([results[c]["y"] for c in range(N_CORES)], axis=0)


def _run(x, weight, bias, trace=False):
    from concourse.bass_utils import run_bass_kernel_spmd

    nc = _build_bass()
    in_maps = _host_inputs(x, weight, bias)
    res = run_bass_kernel_spmd(nc, in_maps, core_ids=list(range(N_CORES)),
                               trace=trace)
    y = _gather_output(res.results)
    return y, res


def kernel(x, weight, bias):
    import time
    try:
        y, _ = _run(x, weight, bias, trace=False)
    except Exception:
        # transient device hiccups (NRT_EXEC_UNIT_UNRECOVERABLE) usually
        # clear on a clean retry
        time.sleep(10)
        y, _ = _run(x, weight, bias, trace=False)
    return y


# revision 42
# speedup vs baseline: 1.1885x; 1.1885x over previous
"""DropConnect dense MLP kernel for Trainium2 (8 NeuronCores, data-parallel).

Computes y[b,o] = sum_i x[b,i] * w[i,o] * m[b,i,o] + bias[o]  where
m = jax.random.bernoulli(jax.random.key(42), 0.5, (128, 1024, 1024)).

The Bernoulli mask is a *constant* of the function (fixed key, fixed shape,
independent of all inputs): it is materialized once with the exact same
jax.random.bernoulli call the reference makes (the stream is backend-dependent
-- rbg/RngBitGenerator -- so the call is made on the default backend, exactly
like the reference) and shipped to the device as int8 {0,1} planes.

Device work per core (16 samples, data-parallel over batch):
  - wm = w * m for 4 samples per DVE tensor_tensor (f32 * int8 -> f32,
    w read through a step-0 broadcast AP)
  - y contributions via PE matmuls: per (sample, k-chunk) a [128,4] zero
    padded lhsT (x chunk in column sample//4) against wm, accumulated in
    PSUM. Samples rotate over the four 32-column PE groups
    (tile_position=(0,32j), j = sample%4) so the fp32 LOW_HIGH streams of
    4 samples overlap in the systolic array.
  - bias added with K=1 matmuls of ones[1,4].T @ bias[1,512] per group.
"""

import numpy as np

B, IN, OUT = 128, 1024, 1024
N_CORES = 8
BPC = B // N_CORES          # 16 samples per core
KCH = IN // 128             # 8 contraction chunks
NQ = BPC // 4               # 4 sample-quads per core

DROP_PROB = 0.5  # prob a weight is KEPT, matching the reference

# Route mask-multiply tiles with index % GPSIMD_MOD == 1 to GPSIMD (POOL).
# Measured SLOWER than DVE-only (SBUF port contention); 0 disables.
GPSIMD_MOD = 0

_mask_cache = None


def _dropconnect_mask_u8():
    """The constant keep-mask as uint8 {0,1}, shape (B, IN, OUT).

    Must match jax.random.bernoulli(jax.random.key(42), ...) bit-for-bit.
    The default PRNG impl here is 'rbg' (XLA RngBitGenerator), whose stream
    is backend-dependent, so this makes the *identical* call the reference
    makes, with no device pinning.
    """
    global _mask_cache
    if _mask_cache is None:
        import jax
        m = jax.random.bernoulli(jax.random.key(42), DROP_PROB, (B, IN, OUT))
        _mask_cache = np.asarray(m).astype(np.uint8)
    return _mask_cache


def _build_bass():
    import concourse.bacc as bacc
    import concourse.bass as bass
    import concourse.mybir as mybir
    import concourse.tile as tile

    nc = bacc.Bacc("TRN2", target_bir_lowering=False, debug=False,
                   num_devices=N_CORES)

    w_d = nc.dram_tensor("w", [KCH, 128, OUT], mybir.dt.float32,
                         kind="ExternalInput")
    xblk_d = nc.dram_tensor("xblk", [128, KCH * BPC * 4], mybir.dt.float32,
                            kind="ExternalInput")
    m8_d = nc.dram_tensor("m8", [KCH, 128, BPC * OUT], mybir.dt.int8,
                          kind="ExternalInput")
    bias_d = nc.dram_tensor("bias", [1, OUT], mybir.dt.float32,
                            kind="ExternalInput")
    ones_d = nc.dram_tensor("ones", [1, 4], mybir.dt.float32,
                            kind="ExternalInput")
    y_d = nc.dram_tensor("y", [BPC, OUT], mybir.dt.float32,
                         kind="ExternalOutput")
    y_ap = y_d.ap().rearrange("(q j) o -> j q o", j=4)

    with tile.TileContext(nc) as tc:
        with (
            tc.tile_pool(name="const", bufs=1) as cpool,
            tc.tile_pool(name="mstream", bufs=4) as mpool,
            tc.tile_pool(name="wm", bufs=5) as wmpool,
            tc.tile_pool(name="out", bufs=1) as opool,
            tc.tile_pool(name="psum", bufs=1, space="PSUM") as ppool,
        ):
            # first mask quad leads on the sync HWDGE ring so TT0 starts
            # early; small constants slot in behind it; W on the ACT ring.
            QSZ = 4 * OUT  # one quad of int8 mask
            # k=0 mask pieces alternate between the sync HWDGE ring and the
            # gpsimd SWDGE ring so the early loads transfer in parallel;
            # small leading pieces cut the first TT's dependency.
            m0_pieces = []  # (b0, nsamp, tile)
            k0_splits = [(0, 4), (4, 4), (8, 4), (12, 4)]
            for idx, (b0, ns) in enumerate(k0_splits):
                m0 = cpool.tile([128, ns * OUT], mybir.dt.int8,
                                name=f"m0p{idx}", tag=f"m0p{idx}")
                nc.sync.dma_start(out=m0[:],
                                  in_=m8_d[0][:, b0 * OUT:(b0 + ns) * OUT])
                m0_pieces.append((b0, ns, m0))
            # xblk/bias/ones ride behind the k=0 mask pieces: the DVE must
            # never wait, while the PE has ~100us of slack to absorb the lag
            xblk_t = cpool.tile([128, KCH * BPC * 4], mybir.dt.float32,
                                tag="xblk")
            nc.sync.dma_start(out=xblk_t[:], in_=xblk_d[:])
            bias_t = cpool.tile([1, OUT], mybir.dt.float32, tag="bias")
            nc.sync.dma_start(out=bias_t[:], in_=bias_d[:])
            ones_t = cpool.tile([1, 4], mybir.dt.float32, tag="ones")
            nc.sync.dma_start(out=ones_t[:], in_=ones_d[:])
            w_ts = []
            for k in range(KCH):
                w_t = cpool.tile([128, OUT], mybir.dt.float32, tag=f"w{k}")
                nc.scalar.dma_start(out=w_t[:], in_=w_d[k])
                w_ts.append(w_t)

            psum_ts = [ppool.tile([128, 512], mybir.dt.float32,
                                  name=f"ps{h}", tag=f"ps{h}")
                       for h in range(2)]
            # zero-fill so the full-width tail copies read only initialized
            # PSUM; runs during the DMA ramp while the DVE is idle
            for h in range(2):
                nc.vector.memset(psum_ts[h][:], 0.0)

            def emit_tile(k, b0, ns, m_ap, g):
                """Mask-multiply ns samples [b0, b0+ns) and their matmuls."""
                wm_t = wmpool.tile([128, ns * OUT], mybir.dt.float32,
                                   name="wm", tag="wm", padded_shape=None)
                w_sl = w_ts[k][:]
                w_bc = bass.AP(
                    tensor=w_sl.tensor, offset=w_sl.offset,
                    ap=[w_sl.ap[0], [0, ns], w_sl.ap[1]])
                eng = (nc.gpsimd if GPSIMD_MOD and g % GPSIMD_MOD == 1
                       else nc.vector)
                eng.tensor_tensor(
                    wm_t[:, :ns * OUT].rearrange("p (i o) -> p i o", i=ns),
                    w_bc,
                    m_ap.rearrange("p (i o) -> p i o", i=ns),
                    mybir.AluOpType.mult)
                for half in range(2):
                    for i in range(ns):
                        b = b0 + i
                        j = b % 4
                        lhsT = xblk_t[:, (k * BPC + b) * 4:
                                      (k * BPC + b + 1) * 4]
                        rhs = wm_t[:, i * OUT + half * 512:
                                   i * OUT + half * 512 + 512]
                        psl = psum_ts[half][32 * j:32 * j + 4, :]
                        nc.tensor.matmul(psl, lhsT, rhs,
                                         start=(k == 0 and b < 4),
                                         stop=(k == KCH - 1 and b >= BPC - 4),
                                         tile_position=(0, 32 * j))
                if k == 0 and b0 + ns == 4:
                    # all four PE groups are started: fold bias in now
                    for half in range(2):
                        for j in range(4):
                            psl = psum_ts[half][32 * j:32 * j + 4, :]
                            nc.tensor.matmul(
                                psl, ones_t[:],
                                bias_t[:, half * 512:half * 512 + 512],
                                start=False, stop=False,
                                tile_position=(0, 32 * j))

            g = 0
            for b0, ns, m0 in m0_pieces:
                emit_tile(0, b0, ns, m0[:], g)
                g += 1
            # k >= 1: m8 streamed in 1MB halves (2 quad-TTs each), halves
            # alternating across the two ring families for bandwidth headroom
            for k in range(1, KCH):
                for h in range(2):
                    m_t = mpool.tile([128, BPC * OUT // 2], mybir.dt.int8,
                                     tag="m")
                    nc.sync.dma_start(
                        out=m_t[:],
                        in_=m8_d[k][:, h * (BPC * OUT // 2):
                                    (h + 1) * (BPC * OUT // 2)])
                    if k == KCH - 1 and h == 1:
                        # final tiles shrink to single samples: less PE work
                        # exposed after the last DVE op, shorter drain
                        emit_tile(k, 8, 4, m_t[:, 0:QSZ], g)
                        g += 1
                        for i in range(4):
                            emit_tile(k, 12 + i, 1,
                                      m_t[:, QSZ + i * OUT:
                                           QSZ + (i + 1) * OUT], g)
                            g += 1
                    else:
                        for mq in range(2):
                            emit_tile(k, (h * 2 + mq) * 4, 4,
                                      m_t[:, mq * QSZ:(mq + 1) * QSZ], g)
                            g += 1

            # one full-width copy per half (garbage partitions copied too --
            # harmless; only valid rows are DMA'd out). ACT and DVE halves
            # read disjoint PSUM banks and run in parallel.
            ysb_t = opool.tile([128, OUT], mybir.dt.float32, tag="ysb")
            nc.scalar.copy(ysb_t[:, 0:512], psum_ts[0][:])
            nc.vector.tensor_copy(ysb_t[:, 512:1024], psum_ts[1][:])
            for j in range(4):
                nc.sync.dma_start(out=y_ap[j],
                                  in_=ysb_t[32 * j:32 * j + 4, :])

    nc.compile()
    return nc


def _host_inputs(x, weight, bias):
    """Per-core input maps (layouts are pure data movement + zero padding)."""
    x = np.asarray(x, np.float32)
    weight = np.asarray(weight, np.float32)
    bias = np.asarray(bias, np.float32).reshape(1, OUT)

    mask = _dropconnect_mask_u8()

    # weight: [IN, OUT] -> [KCH, 128, OUT]
    w_host = np.ascontiguousarray(weight.reshape(KCH, 128, OUT))
    ones = np.ones((1, 4), np.float32)

    in_maps = []
    for c in range(N_CORES):
        xs = x[c * BPC:(c + 1) * BPC]  # [BPC, IN]
        xblk = np.zeros((128, KCH, BPC, 4), np.float32)
        for k in range(KCH):
            for b in range(BPC):
                xblk[:, k, b, b // 4] = xs[b, k * 128:(k + 1) * 128]
        xblk = xblk.reshape(128, KCH * BPC * 4)

        msk = mask[c * BPC:(c + 1) * BPC]  # [BPC, IN, OUT] uint8
        m8 = np.ascontiguousarray(
            msk.reshape(BPC, KCH, 128, OUT).transpose(1, 2, 0, 3)
        ).reshape(KCH, 128, BPC * OUT).view(np.int8)

        in_maps.append({
            "w": w_host, "xblk": xblk, "m8": m8,
            "bias": bias, "ones": ones,
        })
    return in_maps


def _gather_output(results):
    # DRAM row q*4+j holds sample b = 4q+j == the row index itself: identity.
    return np.concatenate([results[c]["y"] for c in range(N_CORES)], axis=0)


def _run(x, weight, bias, trace=False):
    from concourse.bass_utils import run_bass_kernel_spmd

    nc = _build_bass()
    in_maps = _host_inputs(x, weight, bias)
    res = run_bass_kernel_spmd(nc, in_maps, core_ids=list(range(N_CORES)),
                               trace=trace)
    y = _gather_output(res.results)
    return y, res


def kernel(x, weight, bias):
    import time
    try:
        y, _ = _run(x, weight, bias, trace=False)
    except Exception:
        # transient device hiccups (NRT_EXEC_UNIT_UNRECOVERABLE) usually
        # clear on a clean retry
        time.sleep(10)
        y, _ = _run(x, weight, bias, trace=False)
    return y
